# revision 29
# baseline (speedup 1.0000x reference)
"""GNN message-passing (scatter-add) kernel for 8 Trainium2 NeuronCores.

Computes out = segment_sum(x[src], dst, num_segments=N) for
x [10000, 128] f32, edge_index [2, 320000] int64.

Strategy — dense count-matrix matmul (no gathers, no collectives):
  out[d] = sum_s A[s, d] * x[s]   with A[s, d] = #edges s->d.

  - Host computes A (np.bincount over (src, dst) pairs) and shards its
    columns: core c owns dst range [c*1264, (c+1)*1264). A entries are
    small ints, exact in fp8e4 (<=16); larger counts split into extra
    passes (never triggers for random graphs).
  - On device, out^T[f, d] = sum_k x_k^T-stationary @ A_k-moving: the
    contraction runs over 79 source-node chunks of 128 on the PE with
    d-tiles of 512|512|240 accumulating f32 into three persistent PSUM
    banks.
  - Each load group owns a contiguous per-group DRAM tensor carrying
    its A bytes and feature bytes, streamed on one HWDGE queue in
    exact PE consumption order.
  - Mixed precision: most chunks run fp16-x-stationary x fp8-A-moving
    (1 col/cycle). FP8_GROUPS run PAIRS of chunks in one DoubleRow
    matmul (fp8 x packed 2-per-PE-cell, fp8 A pairs on 2 XBUSes),
    contracting 256 sources per pass — ~1.8x PE throughput on those
    chunks at ~1.3e-2 relative L2 (quantization of the fp8 x share),
    within the 2e-2 gate. A stays exact.
  - PE warmup matmuls run during the first load's DMA window so the
    HAM clock-gate is open (2.4 GHz) before real data arrives; the
    warm window must be >= 3.4us of continuous busy.
  - The last DRAIN_CHUNKS chunks run tile-major so each PSUM d-tile
    finishes and drains (DVE copy + out DMA on the second HWDGE
    queue) under the remaining matmuls of the other tiles.
  - Host transposes/concats the 8 cores' out^T tiles back to
    [10000, 128].

Per-core traffic: ~15.2MB stream + 0.65MB out.
"""

import sys

for _p in ("/opt/trn_rl_repo",):
    if _p not in sys.path:
        sys.path.append(_p)

import ml_dtypes
import numpy as np

import concourse.bacc as bacc
import concourse.mybir as mybir
import concourse.tile as tile
from concourse.bass_utils import run_bass_kernel_spmd

N_NODES = 10000
D_FEAT = 128
N_CORES = 8
P = 128
KCH = -(-N_NODES // P)  # 79 source chunks
NPAD = KCH * P  # 10112 (source rows padded; dst needs no padding)
DCORE = NPAD // N_CORES  # 1264 dst columns per core (16B-aligned A rows)
XB = D_FEAT * 2  # 256 bytes of fp16 features per chunk-partition
CHB = DCORE + XB  # 1520 bytes per chunk per partition (fp16 groups)
DTILES = [(0, 512), (512, 512), (1024, 176), (1200, DCORE - 1200)]
# ('g', n): n fp16 chunks; ('m', 3): one fp8 DoubleRow pair + one fp16
# chunk (the fp16 matmuls hide the pair's FWL-less LDWEIGHTS).
KGROUPS = [("g", 3), ("g", 4), ("g", 4)] + [("m", 6)] * 7 + [("g", 4)] * 6 + [("g", 2)]  # 79
DRAIN_CHUNKS = 7  # tile-major drain spans the last two groups
FP8 = ml_dtypes.float8_e4m3
FP8_MAX_INT = 16
N_WARMUP = 8  # dummy PE warmup matmuls (512 cols each)

# test/profiling hooks
TRACE = False
TRACE_CORES = None
LAST_RESULT = None


PAIRB = 2 * DCORE + 2 * D_FEAT  # bytes/partition of one DoubleRow pair


def _group_layout(n_passes: int):
    """(gi, k0, gn, kind, nbytes-per-partition) per group. Multi-pass
    inputs (counts > 16; never for random graphs) fall back to all-fp16."""
    gk = []
    k0 = 0
    for gi, (kind, gn) in enumerate(KGROUPS):
        if n_passes > 1:
            kind = "g"
        nb = (gn // 3) * (PAIRB + CHB) if kind == "m" else gn * CHB
        gk.append((gi, k0, gn, kind, nb))
        k0 += gn
    return gk


def _build_program(n_passes: int):
    nc = bacc.Bacc(
        "TRN2", target_bir_lowering=False, debug=False, num_devices=N_CORES
    )
    gk = _group_layout(n_passes)
    a_ds = {
        (ip, gi): nc.dram_tensor(
            f"a{ip}g{gi}", [P, nb], mybir.dt.float8e4, kind="ExternalInput"
        )
        for ip in range(n_passes)
        for gi, k0, gn, kind, nb in gk
    }
    o_d = nc.dram_tensor("o", [P, DCORE], mybir.dt.float32, kind="ExternalOutput")

    import contextlib

    with tile.TileContext(nc) as tc:
        with contextlib.ExitStack() as stack:
            wp = stack.enter_context(tc.tile_pool(name="warm", bufs=1))
            pools = {}
            for gi, k0, gn, kind, nb in gk:
                key = (kind, gn)
                if key not in pools:
                    bufs = 10 if key == ("g", 4) else 6 if kind == "m" else 2
                    pools[key] = stack.enter_context(
                        tc.tile_pool(name=f"{kind}{gn}", bufs=bufs)
                    )
            resp = stack.enter_context(tc.tile_pool(name="res", bufs=4))
            psp = stack.enter_context(
                tc.tile_pool(name="ps", bufs=1, space="PSUM")
            )

            # PE warmup: dummy matmuls on a zeroed scratch tile into a
            # scratch PSUM bank, so the HAM clock-gate opens during the
            # first loads' DMA window (needs >=3.4us continuous busy).
            warm = wp.tile([P, 512], mybir.dt.float16, tag="warm", name="warm")
            wps = psp.tile([P, 512], mybir.dt.float32, tag="wps", name="wps")
            nc.gpsimd.memset(warm[:], 0.0)
            # prewarm the second HWDGE ring so the drain DMAs at the end
            # don't pay first-use latency
            pre = wp.tile([P, 64], mybir.dt.float8e4, tag="pre", name="pre")
            nc.scalar.dma_start(out=pre[:], in_=a_ds[(0, 0)][:, 0:64])
            for _ in range(N_WARMUP):
                nc.tensor.matmul(
                    wps[:], warm[:, 0:P], warm[:], start=True, stop=True
                )
            pss = [
                psp.tile([P, w], mybir.dt.float32, tag=f"ps{t}", name=f"ps{t}")
                for t, (off, w) in enumerate(DTILES)
            ]
            mi = 0
            n_mm = n_passes * KCH
            drain_mms = []  # (a_sb tile, kk) pairs for the tail
            for ip in range(n_passes):
                for gi, k0, gn, kind, nb in gk:
                    a_sb = pools[(kind, gn)].tile(
                        [P, nb],
                        mybir.dt.float8e4,
                        tag=f"{kind}{gn}",
                        name=f"a{ip}_{gi}",
                    )
                    nc.sync.dma_start(out=a_sb[:], in_=a_ds[(ip, gi)][:])
                    if kind == "m":
                        for j in range(gn // 3):
                            base = j * (PAIRB + CHB)
                            av = a_sb[:, base : base + 2 * DCORE].rearrange(
                                "p (two d) -> p two d", two=2, d=DCORE
                            )
                            xv = a_sb[
                                :, base + 2 * DCORE : base + PAIRB
                            ].rearrange("p (two f) -> p two f", two=2, f=D_FEAT)
                            for t, (off, w) in enumerate(DTILES):
                                nc.tensor.matmul(
                                    pss[t][:],
                                    xv[:],
                                    av[:, :, off : off + w],
                                    start=False,
                                    stop=False,
                                    perf_mode=mybir.MatmulPerfMode.DoubleRow,
                                )
                            mi += 2
                            # trailing fp16 chunk: its FWL matmuls cover
                            # the next pair's DoubleRow LDWEIGHTS
                            cb = base + PAIRB
                            cv = a_sb[:, cb : cb + CHB].rearrange(
                                "p (k c) -> p k c", k=1, c=CHB
                            )
                            xt = cv[:, 0, DCORE:CHB].bitcast(
                                mybir.dt.float16
                            )
                            for t, (off, w) in enumerate(DTILES):
                                nc.tensor.matmul(
                                    pss[t][:],
                                    xt,
                                    cv[:, 0, off : off + w],
                                    start=False,
                                    stop=False,
                                )
                            mi += 1
                        continue
                    avg = a_sb[:].rearrange("p (k c) -> p k c", k=gn, c=CHB)
                    for kk in range(gn):
                        if ip == n_passes - 1 and mi >= n_mm - DRAIN_CHUNKS:
                            drain_mms.append((avg, kk))
                            mi += 1
                            continue
                        xt = avg[:, kk, DCORE:CHB].bitcast(mybir.dt.float16)
                        for t, (off, w) in enumerate(DTILES):
                            nc.tensor.matmul(
                                pss[t][:],
                                xt,
                                avg[:, kk, off : off + w],
                                start=(mi == 0),
                                stop=False,
                            )
                        mi += 1
            # tile-major tail over the final DRAIN_CHUNKS chunks: each PSUM
            # d-tile finishes and drains (DVE copy + out DMA) under the
            # remaining matmuls of the other tiles
            for t, (off, w) in enumerate(DTILES):
                for j, (avg, kk) in enumerate(drain_mms):
                    xt = avg[:, kk, DCORE:CHB].bitcast(mybir.dt.float16)
                    nc.tensor.matmul(
                        pss[t][:],
                        xt,
                        avg[:, kk, off : off + w],
                        start=False,
                        stop=(j == len(drain_mms) - 1),
                    )
                res = resp.tile(
                    [P, w], mybir.dt.float32, tag=f"res{t}", name=f"res{t}"
                )
                nc.vector.tensor_copy(res[:], pss[t][:])
                eng = nc.scalar if t % 2 == 0 else nc.sync
                eng.dma_start(out=o_d[:, off : off + w], in_=res[:])

    nc.compile()
    return nc


def _prepare(x: np.ndarray, edge_index: np.ndarray):
    ei = np.asarray(edge_index)  # pull to host before any indexing
    src = ei[0].astype(np.int64)
    dst = ei[1].astype(np.int64)

    xf = np.asarray(x).astype(np.float32)
    xp = np.zeros((NPAD, D_FEAT), np.float32)
    xp[:N_NODES] = xf
    # per-chunk feature bytes: fp16 for normal groups, fp8 for pair groups
    x16 = (
        np.ascontiguousarray(
            xp.astype(np.float16).reshape(KCH, P, D_FEAT).transpose(1, 0, 2)
        )
        .view(np.uint8)
        .reshape(P, KCH, XB)
    )
    x8 = (
        np.ascontiguousarray(
            xp.astype(FP8).reshape(KCH, P, D_FEAT).transpose(1, 0, 2)
        )
        .view(np.uint8)
        .reshape(P, KCH, D_FEAT)
    )

    # per-core count matrices and pass counts first
    core_cnts = []
    n_passes = 1
    for c in range(N_CORES):
        sel = (dst >= c * DCORE) & (dst < (c + 1) * DCORE)
        idx = src[sel] * DCORE + (dst[sel] - c * DCORE)
        cnt = np.bincount(idx, minlength=NPAD * DCORE).reshape(NPAD, DCORE)
        core_cnts.append(cnt)
        mx = int(cnt.max())
        need = 1 if mx == 0 else -(-mx // FP8_MAX_INT)
        n_passes = max(n_passes, need)

    gk = _group_layout(n_passes)
    in_maps = [{} for _ in range(N_CORES)]
    for c in range(N_CORES):
        cnt = core_cnts[c]
        for ip in range(n_passes):
            part = np.minimum(cnt, FP8_MAX_INT)
            ab = (
                np.ascontiguousarray(
                    part.astype(FP8).reshape(KCH, P, DCORE).transpose(1, 0, 2)
                )
                .view(np.uint8)
                .reshape(P, KCH, DCORE)
            )
            cnt = cnt - part
            for gi, k0, gn, kind, nb in gk:
                buf = np.empty((P, nb), np.uint8)
                if kind == "m":
                    # per triple: [A pair][fp8 x pair][fp16 chunk: A + x16]
                    for j in range(gn // 3):
                        b = j * (PAIRB + CHB)
                        kj = k0 + 3 * j
                        buf[:, b : b + 2 * DCORE] = ab[
                            :, kj : kj + 2, :
                        ].reshape(P, 2 * DCORE)
                        buf[:, b + 2 * DCORE : b + PAIRB] = x8[
                            :, kj : kj + 2, :
                        ].reshape(P, 2 * D_FEAT)
                        buf[:, b + PAIRB : b + PAIRB + DCORE] = ab[:, kj + 2, :]
                        buf[:, b + PAIRB + DCORE : b + PAIRB + CHB] = x16[
                            :, kj + 2, :
                        ]
                else:
                    v = buf.reshape(P, gn, CHB)
                    v[:, :, :DCORE] = ab[:, k0 : k0 + gn, :]
                    v[:, :, DCORE:] = x16[:, k0 : k0 + gn, :]
                in_maps[c][f"a{ip}g{gi}"] = buf.view(FP8)
    return in_maps, n_passes


def kernel(x: np.ndarray, edge_index: np.ndarray) -> np.ndarray:
    global LAST_RESULT
    in_maps, n_passes = _prepare(x, edge_index)
    nc = _build_program(n_passes)
    res = run_bass_kernel_spmd(
        nc,
        in_maps,
        list(range(N_CORES)),
        trace=TRACE,
        trace_cores=TRACE_CORES if TRACE else None,
    )
    LAST_RESULT = res
    # o per core: [128 f, DCORE d] -> out[c*DCORE + d, f]
    out = np.concatenate(
        [np.asarray(r["o"], np.float32).T for r in res.results], axis=0
    )
    return np.ascontiguousarray(out[:N_NODES])


if __name__ == "__main__":
    rng = np.random.default_rng(0)
    x = rng.standard_normal((N_NODES, D_FEAT), dtype=np.float32)
    edge_index = rng.integers(0, N_NODES, size=(2, 320000)).astype(np.int64)
    out = kernel(x, edge_index)
    ref = np.zeros((N_NODES, D_FEAT), np.float32)
    np.add.at(ref, edge_index[1], x[edge_index[0]])
    rel = np.linalg.norm(out - ref) / np.linalg.norm(ref)
    print("rel L2 err:", rel)


# revision 30
# speedup vs baseline: 1.1022x; 1.1022x over previous
"""GNN message-passing (scatter-add) kernel for 8 Trainium2 NeuronCores.

Computes out = segment_sum(x[src], dst, num_segments=N) for
x [10000, 128] f32, edge_index [2, 320000] int64.

Strategy — dense count-matrix matmul (no gathers, no collectives):
  out[d] = sum_s A[s, d] * x[s]   with A[s, d] = #edges s->d.

  - Host computes A (np.bincount over (src, dst) pairs) and shards its
    columns: core c owns dst range [c*1264, (c+1)*1264). A entries are
    small ints, exact in fp8e4 (<=16); larger counts split into extra
    passes (never triggers for random graphs).
  - On device, out^T[f, d] = sum_k x_k^T-stationary @ A_k-moving: the
    contraction runs over 79 source-node chunks of 128 on the PE with
    d-tiles of 512|512|240 accumulating f32 into three persistent PSUM
    banks.
  - Each load group owns a contiguous per-group DRAM tensor carrying
    its A bytes and feature bytes, streamed on one HWDGE queue in
    exact PE consumption order.
  - Mixed precision: most chunks run fp16-x-stationary x fp8-A-moving
    (1 col/cycle). FP8_GROUPS run PAIRS of chunks in one DoubleRow
    matmul (fp8 x packed 2-per-PE-cell, fp8 A pairs on 2 XBUSes),
    contracting 256 sources per pass — ~1.8x PE throughput on those
    chunks at ~1.3e-2 relative L2 (quantization of the fp8 x share),
    within the 2e-2 gate. A stays exact.
  - PE warmup matmuls run during the first load's DMA window so the
    HAM clock-gate is open (2.4 GHz) before real data arrives; the
    warm window must be >= 3.4us of continuous busy.
  - The last DRAIN_CHUNKS chunks run tile-major so each PSUM d-tile
    finishes and drains (DVE copy + out DMA on the second HWDGE
    queue) under the remaining matmuls of the other tiles.
  - Host transposes/concats the 8 cores' out^T tiles back to
    [10000, 128].

Per-core traffic: ~15.2MB stream + 0.65MB out.
"""

import sys

for _p in ("/opt/trn_rl_repo",):
    if _p not in sys.path:
        sys.path.append(_p)

import ml_dtypes
import numpy as np

import concourse.bacc as bacc
import concourse.mybir as mybir
import concourse.tile as tile
from concourse.bass_utils import run_bass_kernel_spmd

N_NODES = 10000
D_FEAT = 128
N_CORES = 8
P = 128
KCH = -(-N_NODES // P)  # 79 source chunks
NPAD = KCH * P  # 10112 (source rows padded; dst needs no padding)
DCORE = NPAD // N_CORES  # 1264 dst columns per core (16B-aligned A rows)
XB = D_FEAT * 2  # 256 bytes of fp16 features per chunk-partition
CHB = DCORE + XB  # 1520 bytes per chunk per partition (fp16 groups)
DTILES = [(0, 512), (512, 512), (1024, 176), (1200, DCORE - 1200)]
# ('g', n): n fp16 chunks; ('m', 3): one fp8 DoubleRow pair + one fp16
# chunk (the fp16 matmuls hide the pair's FWL-less LDWEIGHTS).
KGROUPS = [("g", 3), ("g", 4)] + [("m", 6)] * 7 + [("g", 4)] * 7 + [("g", 2)]  # 79
DRAIN_CHUNKS = 7  # tile-major drain spans the last two groups
FP8 = ml_dtypes.float8_e4m3
FP8_MAX_INT = 16
N_WARMUP = 8  # dummy PE warmup matmuls (512 cols each)

# test/profiling hooks
TRACE = False
TRACE_CORES = None
LAST_RESULT = None


PAIRB = 2 * DCORE + 2 * D_FEAT  # bytes/partition of one DoubleRow pair


def _group_layout(n_passes: int):
    """(gi, k0, gn, kind, nbytes-per-partition) per group. Multi-pass
    inputs (counts > 16; never for random graphs) fall back to all-fp16."""
    gk = []
    k0 = 0
    for gi, (kind, gn) in enumerate(KGROUPS):
        if n_passes > 1:
            kind = "g"
        nb = (gn // 3) * (PAIRB + CHB) if kind == "m" else gn * CHB
        gk.append((gi, k0, gn, kind, nb))
        k0 += gn
    return gk


def _build_program(n_passes: int):
    nc = bacc.Bacc(
        "TRN2", target_bir_lowering=False, debug=False, num_devices=N_CORES
    )
    gk = _group_layout(n_passes)
    a_ds = {
        (ip, gi): nc.dram_tensor(
            f"a{ip}g{gi}", [P, nb], mybir.dt.float8e4, kind="ExternalInput"
        )
        for ip in range(n_passes)
        for gi, k0, gn, kind, nb in gk
    }
    o_d = nc.dram_tensor("o", [P, DCORE], mybir.dt.float32, kind="ExternalOutput")

    import contextlib

    with tile.TileContext(nc) as tc:
        with contextlib.ExitStack() as stack:
            wp = stack.enter_context(tc.tile_pool(name="warm", bufs=1))
            pools = {}
            for gi, k0, gn, kind, nb in gk:
                key = (kind, gn)
                if key not in pools:
                    bufs = 10 if key == ("g", 4) else 6 if kind == "m" else 2
                    pools[key] = stack.enter_context(
                        tc.tile_pool(name=f"{kind}{gn}", bufs=bufs)
                    )
            resp = stack.enter_context(tc.tile_pool(name="res", bufs=4))
            psp = stack.enter_context(
                tc.tile_pool(name="ps", bufs=1, space="PSUM")
            )

            # PE warmup: dummy matmuls on a zeroed scratch tile into a
            # scratch PSUM bank, so the HAM clock-gate opens during the
            # first loads' DMA window (needs >=3.4us continuous busy).
            warm = wp.tile([P, 512], mybir.dt.float16, tag="warm", name="warm")
            wps = psp.tile([P, 512], mybir.dt.float32, tag="wps", name="wps")
            nc.gpsimd.memset(warm[:], 0.0)
            # prewarm the second HWDGE ring so the drain DMAs at the end
            # don't pay first-use latency
            pre = wp.tile([P, 64], mybir.dt.float8e4, tag="pre", name="pre")
            nc.scalar.dma_start(out=pre[:], in_=a_ds[(0, 0)][:, 0:64])
            for _ in range(N_WARMUP):
                nc.tensor.matmul(
                    wps[:], warm[:, 0:P], warm[:], start=True, stop=True
                )
            pss = [
                psp.tile([P, w], mybir.dt.float32, tag=f"ps{t}", name=f"ps{t}")
                for t, (off, w) in enumerate(DTILES)
            ]
            mi = 0
            n_mm = n_passes * KCH
            drain_mms = []  # (a_sb tile, kk) pairs for the tail
            for ip in range(n_passes):
                for gi, k0, gn, kind, nb in gk:
                    a_sb = pools[(kind, gn)].tile(
                        [P, nb],
                        mybir.dt.float8e4,
                        tag=f"{kind}{gn}",
                        name=f"a{ip}_{gi}",
                    )
                    nc.sync.dma_start(out=a_sb[:], in_=a_ds[(ip, gi)][:])
                    if kind == "m":
                        for j in range(gn // 3):
                            base = j * (PAIRB + CHB)
                            av = a_sb[:, base : base + 2 * DCORE].rearrange(
                                "p (two d) -> p two d", two=2, d=DCORE
                            )
                            xv = a_sb[
                                :, base + 2 * DCORE : base + PAIRB
                            ].rearrange("p (two f) -> p two f", two=2, f=D_FEAT)
                            for t, (off, w) in enumerate(DTILES):
                                nc.tensor.matmul(
                                    pss[t][:],
                                    xv[:],
                                    av[:, :, off : off + w],
                                    start=False,
                                    stop=False,
                                    perf_mode=mybir.MatmulPerfMode.DoubleRow,
                                )
                            mi += 2
                            # trailing fp16 chunk: its FWL matmuls cover
                            # the next pair's DoubleRow LDWEIGHTS
                            cb = base + PAIRB
                            cv = a_sb[:, cb : cb + CHB].rearrange(
                                "p (k c) -> p k c", k=1, c=CHB
                            )
                            xt = cv[:, 0, DCORE:CHB].bitcast(
                                mybir.dt.float16
                            )
                            for t, (off, w) in enumerate(DTILES):
                                nc.tensor.matmul(
                                    pss[t][:],
                                    xt,
                                    cv[:, 0, off : off + w],
                                    start=False,
                                    stop=False,
                                )
                            mi += 1
                        continue
                    avg = a_sb[:].rearrange("p (k c) -> p k c", k=gn, c=CHB)
                    for kk in range(gn):
                        if ip == n_passes - 1 and mi >= n_mm - DRAIN_CHUNKS:
                            drain_mms.append((avg, kk))
                            mi += 1
                            continue
                        xt = avg[:, kk, DCORE:CHB].bitcast(mybir.dt.float16)
                        for t, (off, w) in enumerate(DTILES):
                            nc.tensor.matmul(
                                pss[t][:],
                                xt,
                                avg[:, kk, off : off + w],
                                start=(mi == 0),
                                stop=False,
                            )
                        mi += 1
            # tile-major tail over the final DRAIN_CHUNKS chunks: each PSUM
            # d-tile finishes and drains (DVE copy + out DMA) under the
            # remaining matmuls of the other tiles
            for t, (off, w) in enumerate(DTILES):
                for j, (avg, kk) in enumerate(drain_mms):
                    xt = avg[:, kk, DCORE:CHB].bitcast(mybir.dt.float16)
                    nc.tensor.matmul(
                        pss[t][:],
                        xt,
                        avg[:, kk, off : off + w],
                        start=False,
                        stop=(j == len(drain_mms) - 1),
                    )
                res = resp.tile(
                    [P, w], mybir.dt.float32, tag=f"res{t}", name=f"res{t}"
                )
                nc.vector.tensor_copy(res[:], pss[t][:])
                eng = nc.scalar if t % 2 == 0 else nc.sync
                eng.dma_start(out=o_d[:, off : off + w], in_=res[:])

    nc.compile()
    return nc


def _prepare(x: np.ndarray, edge_index: np.ndarray):
    ei = np.asarray(edge_index)  # pull to host before any indexing
    src = ei[0].astype(np.int64)
    dst = ei[1].astype(np.int64)

    xf = np.asarray(x).astype(np.float32)
    xp = np.zeros((NPAD, D_FEAT), np.float32)
    xp[:N_NODES] = xf
    # per-chunk feature bytes: fp16 for normal groups, fp8 for pair groups
    x16 = (
        np.ascontiguousarray(
            xp.astype(np.float16).reshape(KCH, P, D_FEAT).transpose(1, 0, 2)
        )
        .view(np.uint8)
        .reshape(P, KCH, XB)
    )
    x8 = (
        np.ascontiguousarray(
            xp.astype(FP8).reshape(KCH, P, D_FEAT).transpose(1, 0, 2)
        )
        .view(np.uint8)
        .reshape(P, KCH, D_FEAT)
    )

    # per-core count matrices and pass counts first
    core_cnts = []
    n_passes = 1
    for c in range(N_CORES):
        sel = (dst >= c * DCORE) & (dst < (c + 1) * DCORE)
        idx = src[sel] * DCORE + (dst[sel] - c * DCORE)
        cnt = np.bincount(idx, minlength=NPAD * DCORE).reshape(NPAD, DCORE)
        core_cnts.append(cnt)
        mx = int(cnt.max())
        need = 1 if mx == 0 else -(-mx // FP8_MAX_INT)
        n_passes = max(n_passes, need)

    gk = _group_layout(n_passes)
    in_maps = [{} for _ in range(N_CORES)]
    for c in range(N_CORES):
        cnt = core_cnts[c]
        for ip in range(n_passes):
            part = np.minimum(cnt, FP8_MAX_INT)
            ab = (
                np.ascontiguousarray(
                    part.astype(FP8).reshape(KCH, P, DCORE).transpose(1, 0, 2)
                )
                .view(np.uint8)
                .reshape(P, KCH, DCORE)
            )
            cnt = cnt - part
            for gi, k0, gn, kind, nb in gk:
                buf = np.empty((P, nb), np.uint8)
                if kind == "m":
                    # per triple: [A pair][fp8 x pair][fp16 chunk: A + x16]
                    for j in range(gn // 3):
                        b = j * (PAIRB + CHB)
                        kj = k0 + 3 * j
                        buf[:, b : b + 2 * DCORE] = ab[
                            :, kj : kj + 2, :
                        ].reshape(P, 2 * DCORE)
                        buf[:, b + 2 * DCORE : b + PAIRB] = x8[
                            :, kj : kj + 2, :
                        ].reshape(P, 2 * D_FEAT)
                        buf[:, b + PAIRB : b + PAIRB + DCORE] = ab[:, kj + 2, :]
                        buf[:, b + PAIRB + DCORE : b + PAIRB + CHB] = x16[
                            :, kj + 2, :
                        ]
                else:
                    v = buf.reshape(P, gn, CHB)
                    v[:, :, :DCORE] = ab[:, k0 : k0 + gn, :]
                    v[:, :, DCORE:] = x16[:, k0 : k0 + gn, :]
                in_maps[c][f"a{ip}g{gi}"] = buf.view(FP8)
    return in_maps, n_passes


def kernel(x: np.ndarray, edge_index: np.ndarray) -> np.ndarray:
    global LAST_RESULT
    in_maps, n_passes = _prepare(x, edge_index)
    nc = _build_program(n_passes)
    res = run_bass_kernel_spmd(
        nc,
        in_maps,
        list(range(N_CORES)),
        trace=TRACE,
        trace_cores=TRACE_CORES if TRACE else None,
    )
    LAST_RESULT = res
    # o per core: [128 f, DCORE d] -> out[c*DCORE + d, f]
    out = np.concatenate(
        [np.asarray(r["o"], np.float32).T for r in res.results], axis=0
    )
    return np.ascontiguousarray(out[:N_NODES])


if __name__ == "__main__":
    rng = np.random.default_rng(0)
    x = rng.standard_normal((N_NODES, D_FEAT), dtype=np.float32)
    edge_index = rng.integers(0, N_NODES, size=(2, 320000)).astype(np.int64)
    out = kernel(x, edge_index)
    ref = np.zeros((N_NODES, D_FEAT), np.float32)
    np.add.at(ref, edge_index[1], x[edge_index[0]])
    rel = np.linalg.norm(out - ref) / np.linalg.norm(ref)
    print("rel L2 err:", rel)


# revision 31
# speedup vs baseline: 1.1114x; 1.0083x over previous
"""GNN message-passing (scatter-add) kernel for 8 Trainium2 NeuronCores.

Computes out = segment_sum(x[src], dst, num_segments=N) for
x [10000, 128] f32, edge_index [2, 320000] int64.

Strategy — dense count-matrix matmul (no gathers, no collectives):
  out[d] = sum_s A[s, d] * x[s]   with A[s, d] = #edges s->d.

  - Host computes A (np.bincount over (src, dst) pairs) and shards its
    columns: core c owns dst range [c*1264, (c+1)*1264). A entries are
    small ints, exact in fp8e4 (<=16); larger counts split into extra
    passes (never triggers for random graphs).
  - On device, out^T[f, d] = sum_k x_k^T-stationary @ A_k-moving: the
    contraction runs over 79 source-node chunks of 128 on the PE with
    d-tiles of 512|512|240 accumulating f32 into three persistent PSUM
    banks.
  - Each load group owns a contiguous per-group DRAM tensor carrying
    its A bytes and feature bytes, streamed on one HWDGE queue in
    exact PE consumption order.
  - Mixed precision: most chunks run fp16-x-stationary x fp8-A-moving
    (1 col/cycle). FP8_GROUPS run PAIRS of chunks in one DoubleRow
    matmul (fp8 x packed 2-per-PE-cell, fp8 A pairs on 2 XBUSes),
    contracting 256 sources per pass — ~1.8x PE throughput on those
    chunks at ~1.3e-2 relative L2 (quantization of the fp8 x share),
    within the 2e-2 gate. A stays exact.
  - PE warmup matmuls run during the first load's DMA window so the
    HAM clock-gate is open (2.4 GHz) before real data arrives; the
    warm window must be >= 3.4us of continuous busy.
  - The last DRAIN_CHUNKS chunks run tile-major so each PSUM d-tile
    finishes and drains (DVE copy + out DMA on the second HWDGE
    queue) under the remaining matmuls of the other tiles.
  - Host transposes/concats the 8 cores' out^T tiles back to
    [10000, 128].

Per-core traffic: ~15.2MB stream + 0.65MB out.
"""

import sys

for _p in ("/opt/trn_rl_repo",):
    if _p not in sys.path:
        sys.path.append(_p)

import ml_dtypes
import numpy as np

import concourse.bacc as bacc
import concourse.mybir as mybir
import concourse.tile as tile
from concourse.bass_utils import run_bass_kernel_spmd

N_NODES = 10000
D_FEAT = 128
N_CORES = 8
P = 128
KCH = -(-N_NODES // P)  # 79 source chunks
NPAD = KCH * P  # 10112 (source rows padded; dst needs no padding)
DCORE = NPAD // N_CORES  # 1264 dst columns per core (16B-aligned A rows)
XB = D_FEAT * 2  # 256 bytes of fp16 features per chunk-partition
CHB = DCORE + XB  # 1520 bytes per chunk per partition (fp16 groups)
DTILES = [(0, 512), (512, 512), (1024, 176), (1200, DCORE - 1200)]
# ('g', n): n fp16 chunks; ('m', 3): one fp8 DoubleRow pair + one fp16
# chunk (the fp16 matmuls hide the pair's FWL-less LDWEIGHTS).
KGROUPS = [("g", 3), ("g", 4)] + [("m", 6)] * 7 + [("g", 4)] * 7 + [("g", 2)]  # 79
DRAIN_CHUNKS = 7  # tile-major drain spans the last two groups
FP8 = ml_dtypes.float8_e4m3
FP8_MAX_INT = 16
N_WARMUP = 8  # dummy PE warmup matmuls (512 cols each)

# test/profiling hooks
TRACE = False
TRACE_CORES = None
LAST_RESULT = None


PAIRB = 2 * DCORE + 2 * D_FEAT  # bytes/partition of one DoubleRow pair


def _group_layout(n_passes: int):
    """(gi, k0, gn, kind, nbytes-per-partition) per group. Multi-pass
    inputs (counts > 16; never for random graphs) fall back to all-fp16."""
    gk = []
    k0 = 0
    for gi, (kind, gn) in enumerate(KGROUPS):
        if n_passes > 1:
            kind = "g"
        nb = (gn // 3) * (PAIRB + CHB) if kind == "m" else gn * CHB
        gk.append((gi, k0, gn, kind, nb))
        k0 += gn
    return gk


def _build_program(n_passes: int):
    nc = bacc.Bacc(
        "TRN2", target_bir_lowering=False, debug=False, num_devices=N_CORES
    )
    gk = _group_layout(n_passes)
    a_ds = {
        (ip, gi): nc.dram_tensor(
            f"a{ip}g{gi}", [P, nb], mybir.dt.float8e4, kind="ExternalInput"
        )
        for ip in range(n_passes)
        for gi, k0, gn, kind, nb in gk
    }
    o_d = nc.dram_tensor("o", [P, DCORE], mybir.dt.float32, kind="ExternalOutput")

    import contextlib

    with tile.TileContext(nc) as tc:
        with contextlib.ExitStack() as stack:
            wp = stack.enter_context(tc.tile_pool(name="warm", bufs=1))
            pools = {}
            for gi, k0, gn, kind, nb in gk:
                key = (kind, gn)
                if key not in pools:
                    bufs = 10 if key == ("g", 4) else 6 if kind == "m" else 2
                    pools[key] = stack.enter_context(
                        tc.tile_pool(name=f"{kind}{gn}", bufs=bufs)
                    )
            resp = stack.enter_context(tc.tile_pool(name="res", bufs=4))
            psp = stack.enter_context(
                tc.tile_pool(name="ps", bufs=1, space="PSUM")
            )

            # PE warmup: dummy matmuls on a zeroed scratch tile into a
            # scratch PSUM bank, so the HAM clock-gate opens during the
            # first loads' DMA window (needs >=3.4us continuous busy).
            warm = wp.tile([P, 512], mybir.dt.float16, tag="warm", name="warm")
            wps = psp.tile([P, 512], mybir.dt.float32, tag="wps", name="wps")
            nc.gpsimd.memset(warm[:], 0.0)
            # prewarm the second HWDGE ring so the drain DMAs at the end
            # don't pay first-use latency
            pre = wp.tile([P, 64], mybir.dt.float8e4, tag="pre", name="pre")
            nc.scalar.dma_start(out=pre[:], in_=a_ds[(0, 0)][:, 0:64])
            for _ in range(N_WARMUP):
                nc.tensor.matmul(
                    wps[:], warm[:, 0:P], warm[:], start=True, stop=True
                )
            pss = [
                psp.tile([P, w], mybir.dt.float32, tag=f"ps{t}", name=f"ps{t}")
                for t, (off, w) in enumerate(DTILES)
            ]
            mi = 0
            n_mm = n_passes * KCH
            drain_mms = []  # (a_sb tile, kk) pairs for the tail
            for ip in range(n_passes):
                for gi, k0, gn, kind, nb in gk:
                    a_sb = pools[(kind, gn)].tile(
                        [P, nb],
                        mybir.dt.float8e4,
                        tag=f"{kind}{gn}",
                        name=f"a{ip}_{gi}",
                    )
                    nc.sync.dma_start(out=a_sb[:], in_=a_ds[(ip, gi)][:])
                    if kind == "m":
                        for j in range(gn // 3):
                            base = j * (PAIRB + CHB)
                            av = a_sb[:, base : base + 2 * DCORE].rearrange(
                                "p (two d) -> p two d", two=2, d=DCORE
                            )
                            xv = a_sb[
                                :, base + 2 * DCORE : base + PAIRB
                            ].rearrange("p (two f) -> p two f", two=2, f=D_FEAT)
                            for t, (off, w) in enumerate(DTILES):
                                nc.tensor.matmul(
                                    pss[t][:],
                                    xv[:],
                                    av[:, :, off : off + w],
                                    start=False,
                                    stop=False,
                                    perf_mode=mybir.MatmulPerfMode.DoubleRow,
                                )
                            mi += 2
                            # trailing fp16 chunk: its FWL matmuls cover
                            # the next pair's DoubleRow LDWEIGHTS
                            cb = base + PAIRB
                            cv = a_sb[:, cb : cb + CHB].rearrange(
                                "p (k c) -> p k c", k=1, c=CHB
                            )
                            xt = cv[:, 0, DCORE:CHB].bitcast(
                                mybir.dt.float16
                            )
                            for t, (off, w) in enumerate(DTILES):
                                nc.tensor.matmul(
                                    pss[t][:],
                                    xt,
                                    cv[:, 0, off : off + w],
                                    start=False,
                                    stop=False,
                                )
                            mi += 1
                        continue
                    avg = a_sb[:].rearrange("p (k c) -> p k c", k=gn, c=CHB)
                    for kk in range(gn):
                        if ip == n_passes - 1 and mi >= n_mm - DRAIN_CHUNKS:
                            drain_mms.append((avg, kk))
                            mi += 1
                            continue
                        xt = avg[:, kk, DCORE:CHB].bitcast(mybir.dt.float16)
                        for t, (off, w) in enumerate(DTILES):
                            nc.tensor.matmul(
                                pss[t][:],
                                xt,
                                avg[:, kk, off : off + w],
                                start=(mi == 0),
                                stop=False,
                            )
                        mi += 1
            # tile-major tail over the final DRAIN_CHUNKS chunks: each PSUM
            # d-tile finishes and drains (DVE copy + out DMA) under the
            # remaining matmuls of the other tiles
            for t, (off, w) in enumerate(DTILES):
                for j, (avg, kk) in enumerate(drain_mms):
                    xt = avg[:, kk, DCORE:CHB].bitcast(mybir.dt.float16)
                    nc.tensor.matmul(
                        pss[t][:],
                        xt,
                        avg[:, kk, off : off + w],
                        start=False,
                        stop=(j == len(drain_mms) - 1),
                    )
                if t < 2:
                    res = resp.tile(
                        [P, w], mybir.dt.float32, tag=f"res{t}", name=f"res{t}"
                    )
                    nc.vector.tensor_copy(res[:], pss[t][:])
                    eng = nc.scalar if t == 0 else nc.sync
                    eng.dma_start(out=o_d[:, off : off + w], in_=res[:])
                else:
                    # tiles 2+3 are adjacent in d: share one res tile and
                    # one out DMA, cutting a serialized 0.6us issue from
                    # the critical tail
                    if t == 2:
                        res23 = resp.tile(
                            [P, DCORE - 1024],
                            mybir.dt.float32,
                            tag="res23",
                            name="res23",
                        )
                    nc.vector.tensor_copy(
                        res23[:, off - 1024 : off - 1024 + w], pss[t][:]
                    )
                    if t == len(DTILES) - 1:
                        nc.scalar.dma_start(
                            out=o_d[:, 1024:DCORE], in_=res23[:]
                        )

    nc.compile()
    return nc


def _prepare(x: np.ndarray, edge_index: np.ndarray):
    ei = np.asarray(edge_index)  # pull to host before any indexing
    src = ei[0].astype(np.int64)
    dst = ei[1].astype(np.int64)

    xf = np.asarray(x).astype(np.float32)
    xp = np.zeros((NPAD, D_FEAT), np.float32)
    xp[:N_NODES] = xf
    # per-chunk feature bytes: fp16 for normal groups, fp8 for pair groups
    x16 = (
        np.ascontiguousarray(
            xp.astype(np.float16).reshape(KCH, P, D_FEAT).transpose(1, 0, 2)
        )
        .view(np.uint8)
        .reshape(P, KCH, XB)
    )
    x8 = (
        np.ascontiguousarray(
            xp.astype(FP8).reshape(KCH, P, D_FEAT).transpose(1, 0, 2)
        )
        .view(np.uint8)
        .reshape(P, KCH, D_FEAT)
    )

    # per-core count matrices and pass counts first
    core_cnts = []
    n_passes = 1
    for c in range(N_CORES):
        sel = (dst >= c * DCORE) & (dst < (c + 1) * DCORE)
        idx = src[sel] * DCORE + (dst[sel] - c * DCORE)
        cnt = np.bincount(idx, minlength=NPAD * DCORE).reshape(NPAD, DCORE)
        core_cnts.append(cnt)
        mx = int(cnt.max())
        need = 1 if mx == 0 else -(-mx // FP8_MAX_INT)
        n_passes = max(n_passes, need)

    gk = _group_layout(n_passes)
    in_maps = [{} for _ in range(N_CORES)]
    for c in range(N_CORES):
        cnt = core_cnts[c]
        for ip in range(n_passes):
            part = np.minimum(cnt, FP8_MAX_INT)
            ab = (
                np.ascontiguousarray(
                    part.astype(FP8).reshape(KCH, P, DCORE).transpose(1, 0, 2)
                )
                .view(np.uint8)
                .reshape(P, KCH, DCORE)
            )
            cnt = cnt - part
            for gi, k0, gn, kind, nb in gk:
                buf = np.empty((P, nb), np.uint8)
                if kind == "m":
                    # per triple: [A pair][fp8 x pair][fp16 chunk: A + x16]
                    for j in range(gn // 3):
                        b = j * (PAIRB + CHB)
                        kj = k0 + 3 * j
                        buf[:, b : b + 2 * DCORE] = ab[
                            :, kj : kj + 2, :
                        ].reshape(P, 2 * DCORE)
                        buf[:, b + 2 * DCORE : b + PAIRB] = x8[
                            :, kj : kj + 2, :
                        ].reshape(P, 2 * D_FEAT)
                        buf[:, b + PAIRB : b + PAIRB + DCORE] = ab[:, kj + 2, :]
                        buf[:, b + PAIRB + DCORE : b + PAIRB + CHB] = x16[
                            :, kj + 2, :
                        ]
                else:
                    v = buf.reshape(P, gn, CHB)
                    v[:, :, :DCORE] = ab[:, k0 : k0 + gn, :]
                    v[:, :, DCORE:] = x16[:, k0 : k0 + gn, :]
                in_maps[c][f"a{ip}g{gi}"] = buf.view(FP8)
    return in_maps, n_passes


def kernel(x: np.ndarray, edge_index: np.ndarray) -> np.ndarray:
    global LAST_RESULT
    in_maps, n_passes = _prepare(x, edge_index)
    nc = _build_program(n_passes)
    res = run_bass_kernel_spmd(
        nc,
        in_maps,
        list(range(N_CORES)),
        trace=TRACE,
        trace_cores=TRACE_CORES if TRACE else None,
    )
    LAST_RESULT = res
    # o per core: [128 f, DCORE d] -> out[c*DCORE + d, f]
    out = np.concatenate(
        [np.asarray(r["o"], np.float32).T for r in res.results], axis=0
    )
    return np.ascontiguousarray(out[:N_NODES])


if __name__ == "__main__":
    rng = np.random.default_rng(0)
    x = rng.standard_normal((N_NODES, D_FEAT), dtype=np.float32)
    edge_index = rng.integers(0, N_NODES, size=(2, 320000)).astype(np.int64)
    out = kernel(x, edge_index)
    ref = np.zeros((N_NODES, D_FEAT), np.float32)
    np.add.at(ref, edge_index[1], x[edge_index[0]])
    rel = np.linalg.norm(out - ref) / np.linalg.norm(ref)
    print("rel L2 err:", rel)
